# revision 11
# baseline (speedup 1.0000x reference)
"""LDS (diagonal linear state space + AR) kernel for 8 Trainium2 cores.

Computation (per batch b):
    uB[t, s]   = sum_d x[t, d] * B[d, s]
    h[t]       = A * h[t-1] + uB[t]          (h[-1] = h0, A diagonal)
    lds[t, o]  = sum_s h[t, s] * C[s, o]
    out[t, o]  = sum_{i<10} sum_d M[o, d, i] * x[t-i, d]  +  lds[t+10, o]

Sharding: data-parallel over batch, 2 batches per core, no collectives.

On-chip layout is [feature, time]:
  - x is host-transposed/padded to xT [2, 2, 128, PAD+T] (b, d_chunk, d, t)
  - uB produced by f32r matmuls into PSUM [128s, 512t]
  - the recurrence runs as tensor_tensor_scan on VectorE, reading uB from
    PSUM and writing hT [128s, T+16] (tail zeroed for the +10 shift)
  - output tiles [128t, 256o] accumulate 8 C-matmuls + 20 AR matmuls in
    PSUM, then DMA straight to HBM (contiguous rows)
"""

import sys

if "/opt/trn_rl_repo" not in sys.path:
    sys.path.insert(0, "/opt/trn_rl_repo")

import numpy as np

import concourse.bass as bass
import concourse.mybir as mybir
from concourse import bacc
from concourse.tile import TileContext

BSZ = 16
SEQ = 2048
D = 256  # input dim
S = 1024  # state dim
O = 256  # output dim
KX = 10
N_CORES = 8
B_PER_CORE = BSZ // N_CORES  # 2

PAD = 16  # left zero-pad on time for the AR taps (needs >= KX-1 = 9)
HPAD = 16  # right zero-pad on h time for the +10 shift (needs >= KX)
TCH = 512  # uB matmul / scan chunk width (= 1 PSUM bank of fp32)
OTCH = 128  # output tile time width (= partition dim of out psum tile)

F32 = mybir.dt.float32
F32R = mybir.dt.float32r

_CACHED = {}


def _build_nc():
    nc = bass.Bass()

    xt_d = nc.dram_tensor("xt", [B_PER_CORE, 2, 128, PAD + SEQ], F32,
                          kind="ExternalInput")
    b_d = nc.dram_tensor("bmat", [2, 128, S], F32, kind="ExternalInput")
    c_d = nc.dram_tensor("cmat", [8, 128, O], F32, kind="ExternalInput")
    m_d = nc.dram_tensor("mmat", [KX, 2, 128, O], F32, kind="ExternalInput")
    ah_d = nc.dram_tensor("ah", [128, 16], F32, kind="ExternalInput")
    z_d = nc.dram_tensor("zt", [128, HPAD], F32, kind="ExternalInput")
    out_d = nc.dram_tensor("out", [B_PER_CORE, SEQ, O], F32,
                           kind="ExternalOutput")

    with TileContext(nc) as tc:
        with tc.tile_pool(name="persist", bufs=1) as persist, \
             tc.tile_pool(name="ht", bufs=10) as ht_pool, \
             tc.tile_pool(name="outsb", bufs=6) as out_sbuf, \
             tc.tile_pool(name="ubps", bufs=4, space="PSUM") as ub_psum, \
             tc.tile_pool(name="outps", bufs=4, space="PSUM") as out_psum:

            # ---- load persistent operands ----
            xt = {}
            for b in range(B_PER_CORE):
                for dch in range(2):
                    t = persist.tile([128, PAD + SEQ], F32R, tag=f"xt{b}{dch}")
                    nc.sync.dma_start(out=t[:], in_=xt_d[b, dch].bitcast(F32R))
                    xt[b, dch] = t
            bmat = {}
            for dch in range(2):
                t = persist.tile([128, S], F32R, tag=f"bm{dch}")
                nc.sync.dma_start(out=t[:], in_=b_d[dch].bitcast(F32R))
                bmat[dch] = t
            cmat = {}
            for sch in range(8):
                t = persist.tile([128, O], F32R, tag=f"cm{sch}")
                nc.sync.dma_start(out=t[:], in_=c_d[sch].bitcast(F32R))
                cmat[sch] = t
            mmat = {}
            for i in range(KX):
                for dch in range(2):
                    t = persist.tile([128, O], F32R, tag=f"mm{i}{dch}")
                    nc.sync.dma_start(out=t[:], in_=m_d[i, dch].bitcast(F32R))
                    mmat[i, dch] = t
            ah = persist.tile([128, 16], F32, tag="ah")
            nc.sync.dma_start(out=ah[:], in_=ah_d[:])

            # ---- per-batch pipeline ----
            for b in range(B_PER_CORE):
                hts = []
                for sch in range(8):
                    ht = ht_pool.tile([128, SEQ + HPAD], F32R, tag="ht")
                    nc.sync.dma_start(out=ht[:, SEQ:],
                                      in_=z_d[:].bitcast(F32R))
                    a_bc = ah[:, sch:sch + 1].broadcast_to([128, TCH])
                    for tch in range(SEQ // TCH):
                        t0 = tch * TCH
                        ub = ub_psum.tile([128, TCH], F32)
                        for dch in range(2):
                            nc.tensor.matmul(
                                out=ub[:],
                                lhsT=bmat[dch][:, sch * 128:(sch + 1) * 128],
                                rhs=xt[b, dch][:, PAD + t0:PAD + t0 + TCH],
                                start=(dch == 0),
                                stop=(dch == 1),
                            )
                        init = (ah[:, 8 + sch:9 + sch] if tch == 0
                                else ht[:, t0 - 1:t0])
                        nc.vector.tensor_tensor_scan(
                            out=ht[:, t0:t0 + TCH],
                            data0=a_bc,
                            data1=ub[:],
                            initial=init,
                            op0=mybir.AluOpType.mult,
                            op1=mybir.AluOpType.add,
                        )
                    hts.append(ht)

                for tch in range(SEQ // OTCH):
                    t0 = tch * OTCH
                    ops = out_psum.tile([128, O], F32)
                    for sch in range(8):
                        nc.tensor.matmul(
                            out=ops[:],
                            lhsT=hts[sch][:, t0 + KX:t0 + KX + OTCH],
                            rhs=cmat[sch][:],
                            start=(sch == 0),
                            stop=False,
                        )
                    for i in range(KX):
                        for dch in range(2):
                            nc.tensor.matmul(
                                out=ops[:],
                                lhsT=xt[b, dch][:, PAD - i + t0:
                                                PAD - i + t0 + OTCH],
                                rhs=mmat[i, dch][:],
                                start=False,
                                stop=(i == KX - 1 and dch == 1),
                            )
                    osb = out_sbuf.tile([128, O], F32)
                    nc.scalar.copy(out=osb[:], in_=ops[:])
                    nc.sync.dma_start(out=out_d[b, t0:t0 + OTCH, :],
                                      in_=osb[:])

    # Matmult (esp. fused-LDW f32r) supports a limited number of HW sync-wait
    # slots; split excess waits into event-semaphore chains the way
    # Bacc.compile() does.
    import bass_rust as _br
    _br.move_matmul_waits_to_ldweights(nc.m)
    _br.generate_event_semaphores(nc)

    return nc


def _prep_core_inputs(inputs, h0, A, B, C, M, core):
    """Host-side shard + layout prep for one core."""
    bs = slice(core * B_PER_CORE, (core + 1) * B_PER_CORE)
    x = inputs[bs]  # [2, T, D]
    xt = np.zeros((B_PER_CORE, 2, 128, PAD + SEQ), np.float32)
    xtr = np.ascontiguousarray(x.transpose(0, 2, 1))  # [2, D, T]
    xt[:, :, :, PAD:] = xtr.reshape(B_PER_CORE, 2, 128, SEQ)

    bmat = np.ascontiguousarray(B.reshape(2, 128, S), np.float32)
    cmat = np.ascontiguousarray(C.reshape(8, 128, O), np.float32)
    # mmat[i, dch, d, o] = M[o, dch*128+d, i]
    mmat = np.ascontiguousarray(
        M.transpose(2, 1, 0).reshape(KX, 2, 128, O), np.float32)
    ah = np.zeros((128, 16), np.float32)
    ah[:, :8] = A.reshape(8, 128).T
    ah[:, 8:] = h0.reshape(8, 128).T
    return {"xt": xt, "bmat": bmat, "cmat": cmat, "mmat": mmat, "ah": ah,
            "zt": np.zeros((128, HPAD), np.float32)}


LAST_RESULT = None


def kernel(inputs, h0, A, B, C, M):
    global LAST_RESULT
    from concourse.bass_utils import run_bass_kernel_spmd

    inputs = np.asarray(inputs, np.float32)
    h0 = np.asarray(h0, np.float32)
    A = np.asarray(A, np.float32)
    B = np.asarray(B, np.float32)
    C = np.asarray(C, np.float32)
    M = np.asarray(M, np.float32)

    if "nc" not in _CACHED:
        _CACHED["nc"] = _build_nc()
    nc = _CACHED["nc"]

    in_maps = [_prep_core_inputs(inputs, h0, A, B, C, M, c)
               for c in range(N_CORES)]
    res = run_bass_kernel_spmd(nc, in_maps, list(range(N_CORES)))
    LAST_RESULT = res
    out = np.concatenate([res.results[c]["out"] for c in range(N_CORES)],
                         axis=0)
    return out
